# revision 9
# baseline (speedup 1.0000x reference)
"""Trainium2 Bass kernel for nn_Attention_14190571946482.

Single-head causal self-attention with the TF band_part mask quirk:
  q = x @ Wq ; k = x @ Wv ; v = x @ Wk        (naming quirk of the original)
  score = q @ k^T / sqrt(D) + NEG*(j >= i)    (diagonal masked too!)
  out   = softmax(score) @ v
Row 0 is fully masked -> its softmax is exactly uniform over all S
positions, i.e. out[0] = mean_s v[s]; computed via a mean-of-x matmul.

Sharding: 8 cores = 4 batches x 2 roles. Core (b, r) computes q-tiles
SLOT_TILES[s] + r (128 rows each) of batch b with causal kk-block
skipping. Both roles run the IDENTICAL program (SPMD); all role-dependent
structure (which q rows, masks, row-0 blend selectors) is carried in the
input data.

Per-slot schedule: NB full 512-wide kk blocks, the last block trimmed to
width W (the causal frontier), big slots first so the kernel tail is a
short slot. The last slot is the "special" one holding global rows
[128r, 128r+128): for role 0 its row 0 is the fully-masked row, fixed up
by blending in mean(v)/S via host-provided selectors.
"""

import sys

sys.path.insert(0, "/opt/trn_rl_repo")

import numpy as np
import ml_dtypes

import concourse.bass as bass
import concourse.bacc as bacc
import concourse.mybir as mybir
from concourse.tile import TileContext
from concourse import bass_utils

BF16 = ml_dtypes.bfloat16

B, S, D, U = 4, 2048, 512, 512
P = 128
NSLOT = 8
NB = [4, 4, 3, 3, 2, 2, 1, 1]        # kk blocks per slot (last is trimmed)
W = [256, 512, 256, 512, 256, 512, 512, 256]  # width of the last block
SLOT_TILES = [12, 14, 8, 10, 4, 6, 2, 0]      # slot s -> q-tile SLOT_TILES[s]+r
SPECIAL = 7                           # slot holding global rows [128r, 128r+128)
MOFF = np.concatenate([[0], np.cumsum(W)]).astype(int)  # mask col offsets
MTOT = int(MOFF[-1])
SCALE = 1.0 / float(np.sqrt(np.float32(D)))

_nc_cache = None


def build_nc():
    global _nc_cache
    if _nc_cache is not None:
        return _nc_cache

    f32 = mybir.dt.float32
    bf16 = mybir.dt.bfloat16

    nc = bacc.Bacc()
    xT_d = nc.declare_dram_parameter("xT", [D, S], bf16, isOutput=False)
    qx_d = nc.declare_dram_parameter("qx", [D, NSLOT * P], bf16, isOutput=False)
    wq_d = nc.declare_dram_parameter("wq", [D, U], bf16, isOutput=False)
    wv_d = nc.declare_dram_parameter("wv", [D, U], bf16, isOutput=False)
    wk_d = nc.declare_dram_parameter("wk", [D, U], bf16, isOutput=False)
    mm_d = nc.declare_dram_parameter("maskmul", [P, MTOT], bf16, isOutput=False)
    id_d = nc.declare_dram_parameter("ident", [P, P], bf16, isOutput=False)
    rs_d = nc.declare_dram_parameter("rsel", [1, 2], f32, isOutput=False)
    se_d = nc.declare_dram_parameter("sume", [P, 1], f32, isOutput=False)
    out_d = nc.declare_dram_parameter("out", [NSLOT * P, U], f32, isOutput=True)

    with TileContext(nc) as tc:
        with (
            tc.tile_pool(name="cst", bufs=1) as cst,
            tc.tile_pool(name="work", bufs=3) as work,
            tc.tile_pool(name="small", bufs=8) as small,
            tc.tile_pool(name="psA", bufs=3, space="PSUM") as psA,
            tc.tile_pool(name="psC", bufs=2, space="PSUM") as psC,
            tc.tile_pool(name="psT", bufs=2, space="PSUM") as psT,
        ):
            # ---- PE warm-up: ~4us of dummy matmuls so HAM un-throttles the
            # clock (1.2 -> 2.4 GHz) while the input DMAs are still running.
            wu = cst.tile([P, 512], bf16, tag="wu")
            nc.vector.memset(wu, 0.0)
            wups = psA.tile([P, 512], f32, tag="blk")
            for _ in range(10):
                nc.tensor.matmul(wups, lhsT=wu[:, :P], rhs=wu, start=True, stop=True)

            # ---- load inputs to SBUF (qT deps first) ----
            wq, qx, xT, wv, wk = [], [], [], [], []
            for d in range(4):
                t = cst.tile([P, U], bf16, tag=f"wq{d}")
                nc.sync.dma_start(out=t, in_=wq_d[d * P:(d + 1) * P, :])
                wq.append(t)
                t = cst.tile([P, NSLOT * P], bf16, tag=f"qx{d}")
                nc.sync.dma_start(out=t, in_=qx_d[d * P:(d + 1) * P, :])
                qx.append(t)
            for d in range(4):
                t = cst.tile([P, S], bf16, tag=f"xT{d}")
                nc.sync.dma_start(out=t, in_=xT_d[d * P:(d + 1) * P, :])
                xT.append(t)
            for name, dram, lst in (("wv", wv_d, wv), ("wk", wk_d, wk)):
                for d in range(4):
                    t = cst.tile([P, U], bf16, tag=f"{name}{d}")
                    nc.sync.dma_start(out=t, in_=dram[d * P:(d + 1) * P, :])
                    lst.append(t)
            maskmul = cst.tile([P, MTOT], bf16, tag="maskmul")
            nc.sync.dma_start(out=maskmul, in_=mm_d[:, :])
            ident = cst.tile([P, P], bf16, tag="ident")
            nc.sync.dma_start(out=ident, in_=id_d[:, :])
            rsel = cst.tile([1, 2], f32, tag="rsel")
            nc.sync.dma_start(out=rsel, in_=rs_d[:, :])
            sume = cst.tile([P, 1], f32, tag="sume")
            nc.sync.dma_start(out=sume, in_=se_d[:, :])

            # ---- phase 1: qT [u, sq], kT [u, s], v [s, u] ----
            qT = [cst.tile([P, NSLOT * P], bf16, tag=f"qT{u}", name=f"qT{u}")
                  for u in range(4)]
            for u in range(4):
                for h in range(2):
                    ps = psA.tile([P, 512], f32, tag="blk")
                    for d in range(4):
                        nc.tensor.matmul(
                            ps,
                            lhsT=wq[d][:, u * P:(u + 1) * P],
                            rhs=qx[d][:, h * 512:(h + 1) * 512],
                            start=(d == 0), stop=(d == 3),
                        )
                    nc.vector.tensor_copy(qT[u][:, h * 512:(h + 1) * 512], ps)

            kT = [cst.tile([P, S], bf16, tag=f"kT{u}", name=f"kT{u}")
                  for u in range(4)]
            for g in range(4):
                for u in range(4):
                    ps = psA.tile([P, 512], f32, tag="blk")
                    for d in range(4):
                        nc.tensor.matmul(
                            ps,
                            lhsT=wv[d][:, u * P:(u + 1) * P],
                            rhs=xT[d][:, g * 512:(g + 1) * 512],
                            start=(d == 0), stop=(d == 3),
                        )
                    nc.vector.tensor_copy(kT[u][:, g * 512:(g + 1) * 512], ps)

            v_sb = [cst.tile([P, U], bf16, tag=f"v{sc}", name=f"v{sc}")
                    for sc in range(16)]
            for sc in range(16):
                ps = psA.tile([P, 512], f32, tag="blk")
                for d in range(4):
                    nc.tensor.matmul(
                        ps,
                        lhsT=xT[d][:, sc * P:(sc + 1) * P],
                        rhs=wk[d],
                        start=(d == 0), stop=(d == 3),
                    )
                nc.vector.tensor_copy(v_sb[sc], ps)

            # ---- mean-of-v (for the fully-masked global row 0) ----
            xs16 = []
            for d in range(4):
                xs = small.tile([P, 1], f32, tag="xs")
                nc.vector.reduce_sum(xs, xT[d], axis=mybir.AxisListType.X)
                x16 = small.tile([P, 1], bf16, tag="xs16")
                nc.vector.tensor_copy(x16, xs)
                xs16.append(x16)
            vm_ps = psT.tile([1, 512], f32, tag="vm", bufs=1)
            for d in range(4):
                nc.tensor.matmul(vm_ps, lhsT=xs16[d], rhs=wk[d],
                                 start=(d == 0), stop=(d == 3))
            vm_sb = cst.tile([1, 512], f32, tag="vm_sb")
            # vm_sb = sum_s v[s, :] * rscale  (rscale = 1/S for role 0, else 0)
            nc.vector.tensor_scalar_mul(vm_sb, vm_ps, rsel[0:1, 1:2])

            # ---- phase 2: attention per slot ----
            for s in range(NSLOT):
                nb, w = NB[s], W[s]
                ctx_ps = psC.tile([P, 512], f32, tag="ctx")
                bsums = []
                last_c = (nb - 1) * 4 + w // P - 1  # last kk chunk index
                for kb in range(nb):
                    bw = 512 if kb < nb - 1 else w
                    sc_ps = psA.tile([P, 512], f32, tag="blk")
                    for u in range(4):
                        nc.tensor.matmul(
                            sc_ps[:, :bw],
                            lhsT=qT[u][:, s * P:(s + 1) * P],
                            rhs=kT[u][:, kb * 512:kb * 512 + bw],
                            start=(u == 0), stop=(u == 3),
                        )
                    attn = work.tile([P, 512], bf16, tag="attn")
                    bsum = small.tile([P, 1], f32, tag="bsum")
                    if kb == nb - 1:
                        raw = work.tile([P, 512], bf16, tag="raw")
                        nc.scalar.activation(
                            raw[:, :bw], sc_ps[:, :bw],
                            mybir.ActivationFunctionType.Exp, scale=SCALE,
                        )
                        msl = maskmul[:, int(MOFF[s]):int(MOFF[s]) + bw]
                        nc.vector.tensor_mul(attn[:, :bw], raw[:, :bw], msl)
                        nc.vector.reduce_sum(bsum, attn[:, :bw],
                                             axis=mybir.AxisListType.X)
                    else:
                        nc.scalar.activation(
                            attn, sc_ps, mybir.ActivationFunctionType.Exp,
                            scale=SCALE, accum_out=bsum,
                        )
                    bsums.append(bsum)
                    for c in range(bw // P):
                        at_ps = psT.tile([P, P], bf16, tag="at")
                        nc.tensor.transpose(at_ps, attn[:, c * P:(c + 1) * P],
                                            ident)
                        at_sb = work.tile([P, P], bf16, tag="ats")
                        nc.vector.tensor_copy(at_sb, at_ps)
                        cc = kb * 4 + c
                        nc.tensor.matmul(
                            ctx_ps,
                            lhsT=at_sb,
                            rhs=v_sb[cc],
                            start=(cc == 0),
                            stop=(cc == last_c),
                        )
                # combine row sums, reciprocal, normalize, store
                rs = bsums[0]
                for extra in bsums[1:]:
                    nrs = small.tile([P, 1], f32, tag="rs")
                    nc.vector.tensor_add(nrs, rs, extra)
                    rs = nrs
                if s == SPECIAL:
                    nrs = small.tile([P, 1], f32, tag="rs")
                    nc.vector.tensor_add(nrs, rs, sume)
                    rs = nrs
                rcp = small.tile([P, 1], f32, tag="rcp")
                nc.vector.reciprocal(rcp, rs)
                ctx_sb = work.tile([P, 512], f32, tag="ctxs")
                nc.scalar.activation(
                    ctx_sb, ctx_ps, mybir.ActivationFunctionType.Copy,
                    scale=rcp,
                )
                if s == SPECIAL:
                    # row 0 of role 0 = mean(v): ctx*rsel + sum(v)*rscale
                    nc.vector.tensor_scalar_mul(ctx_sb[0:1, :], ctx_sb[0:1, :],
                                                rsel[0:1, 0:1])
                    nc.vector.tensor_add(ctx_sb[0:1, :], ctx_sb[0:1, :], vm_sb)
                nc.sync.dma_start(out=out_d[s * P:(s + 1) * P, :], in_=ctx_sb)

    nc.compile()
    _nc_cache = nc
    return nc


def host_inputs(query, Wq, Wv, Wk):
    """Build per-core input maps. query [B,S,D] f32; W* [D,U] f32."""
    wq16 = Wq.astype(BF16)
    wv16 = Wv.astype(BF16)
    wk16 = Wk.astype(BF16)
    ident = np.eye(P, dtype=BF16)

    i = np.arange(P)[:, None]
    masks = {}
    for r in range(2):
        mm = np.zeros((P, MTOT), np.float32)
        for s in range(NSLOT):
            t = SLOT_TILES[s] + r
            kb = NB[s] - 1
            j = np.arange(W[s])[None, :]
            mm[:, MOFF[s]:MOFF[s] + W[s]] = (512 * kb + j < 128 * t + i)
        masks[r] = mm.astype(BF16)

    in_maps = []
    for core in range(8):
        b, r = core // 2, core % 2
        xTb = np.ascontiguousarray(query[b].T).astype(BF16)       # [D, S]
        cols = np.concatenate(
            [np.arange(128 * (SLOT_TILES[s] + r), 128 * (SLOT_TILES[s] + r) + P)
             for s in range(NSLOT)]
        )
        qx = np.ascontiguousarray(xTb[:, cols])                    # [D, 1024]
        rsel = np.array([[0.0, 1.0 / S]] if r == 0 else [[1.0, 0.0]], np.float32)
        sume = np.zeros((P, 1), np.float32)
        if r == 0:
            sume[0, 0] = 1.0  # avoid 1/0 on the fully-masked row
        in_maps.append({
            "xT": xTb, "qx": qx,
            "wq": wq16, "wv": wv16, "wk": wk16,
            "maskmul": masks[r], "ident": ident,
            "rsel": rsel, "sume": sume,
        })
    return in_maps


def assemble_output(results):
    """results: list of 8 dicts with 'out' [1024, 512] f32."""
    out = np.zeros((B, S, U), np.float32)
    for core in range(8):
        b, r = core // 2, core % 2
        o = np.asarray(results[core]["out"], dtype=np.float32)
        for s in range(NSLOT):
            t = SLOT_TILES[s] + r
            out[b, 128 * t:128 * (t + 1), :] = o[128 * s:128 * (s + 1), :]
    return out


def run(query, Wq, Wv, Wk, **kwargs):
    """Build, compile, and execute on all 8 cores. Returns (output, results)."""
    nc = build_nc()
    in_maps = host_inputs(
        np.asarray(query, np.float32), np.asarray(Wq, np.float32),
        np.asarray(Wv, np.float32), np.asarray(Wk, np.float32),
    )
    res = bass_utils.run_bass_kernel_spmd(nc, in_maps, list(range(8)), **kwargs)
    return assemble_output(res.results), res


def kernel(query, Wq, Wv, Wk):
    out, _ = run(query, Wq, Wv, Wk)
    return out


if __name__ == "__main__":
    rng = np.random.default_rng(0)
    q = rng.standard_normal((B, S, D), dtype=np.float32)
    scale = np.sqrt(2.0 / (D + U)).astype(np.float32)
    Wq = rng.standard_normal((D, U), dtype=np.float32) * scale
    Wv = rng.standard_normal((D, U), dtype=np.float32) * scale
    Wk = rng.standard_normal((D, U), dtype=np.float32) * scale
    out = kernel(q, Wq, Wv, Wk)
    print(out.shape, out.dtype, np.abs(out).mean())


# revision 10
# speedup vs baseline: 1.0254x; 1.0254x over previous
"""Trainium2 Bass kernel for nn_Attention_14190571946482.

Single-head causal self-attention with the TF band_part mask quirk:
  q = x @ Wq ; k = x @ Wv ; v = x @ Wk        (naming quirk of the original)
  score = q @ k^T / sqrt(D) + NEG*(j >= i)    (diagonal masked too!)
  out   = softmax(score) @ v
Row 0 is fully masked -> its softmax is exactly uniform over all S
positions, i.e. out[0] = mean_s v[s]; computed via a mean-of-x matmul.

Sharding: 8 cores = 4 batches x 2 roles. Core (b, r) computes q-tiles
SLOT_TILES[s] + r (128 rows each) of batch b with causal kk-block
skipping. Both roles run the IDENTICAL program (SPMD); all role-dependent
structure (which q rows, masks, row-0 blend selectors) is carried in the
input data.

Per-slot schedule: NB full 512-wide kk blocks, the last block trimmed to
width W (the causal frontier), big slots first so the kernel tail is a
short slot. The last slot is the "special" one holding global rows
[128r, 128r+128): for role 0 its row 0 is the fully-masked row, fixed up
by blending in mean(v)/S via host-provided selectors.
"""

import sys

sys.path.insert(0, "/opt/trn_rl_repo")

import numpy as np
import ml_dtypes

import concourse.bass as bass
import concourse.bacc as bacc
import concourse.mybir as mybir
from concourse.tile import TileContext
from concourse import bass_utils

BF16 = ml_dtypes.bfloat16

B, S, D, U = 4, 2048, 512, 512
P = 128
NSLOT = 8
NB = [4, 4, 3, 3, 2, 2, 1, 1]        # kk blocks per slot (last is trimmed)
W = [256, 512, 256, 512, 256, 512, 512, 256]  # width of the last block
SLOT_TILES = [12, 14, 8, 10, 4, 6, 2, 0]      # slot s -> q-tile SLOT_TILES[s]+r
SPECIAL = 7                           # slot holding global rows [128r, 128r+128)
MOFF = np.concatenate([[0], np.cumsum(W)]).astype(int)  # mask col offsets
MTOT = int(MOFF[-1])
SCALE = 1.0 / float(np.sqrt(np.float32(D)))

_nc_cache = None


def build_nc():
    global _nc_cache
    if _nc_cache is not None:
        return _nc_cache

    f32 = mybir.dt.float32
    bf16 = mybir.dt.bfloat16

    nc = bacc.Bacc()
    xT_d = nc.declare_dram_parameter("xT", [D, S], bf16, isOutput=False)
    qx_d = nc.declare_dram_parameter("qx", [D, NSLOT * P], bf16, isOutput=False)
    wq_d = nc.declare_dram_parameter("wq", [D, U], bf16, isOutput=False)
    wv_d = nc.declare_dram_parameter("wv", [D, U], bf16, isOutput=False)
    wk_d = nc.declare_dram_parameter("wk", [D, U], bf16, isOutput=False)
    mm_d = nc.declare_dram_parameter("maskmul", [P, MTOT], bf16, isOutput=False)
    id_d = nc.declare_dram_parameter("ident", [P, P], bf16, isOutput=False)
    rs_d = nc.declare_dram_parameter("rsel", [1, 2], f32, isOutput=False)
    se_d = nc.declare_dram_parameter("sume", [P, 1], f32, isOutput=False)
    out_d = nc.declare_dram_parameter("out", [NSLOT * P, U], f32, isOutput=True)

    with TileContext(nc) as tc:
        with (
            tc.tile_pool(name="cst", bufs=1) as cst,
            tc.tile_pool(name="work", bufs=3) as work,
            tc.tile_pool(name="small", bufs=8) as small,
            tc.tile_pool(name="psA", bufs=3, space="PSUM") as psA,
            tc.tile_pool(name="psC", bufs=2, space="PSUM") as psC,
            tc.tile_pool(name="psT", bufs=2, space="PSUM") as psT,
        ):
            # ---- PE warm-up: ~4us of dummy matmuls so HAM un-throttles the
            # clock (1.2 -> 2.4 GHz) while the input DMAs are still running.
            wu = cst.tile([P, 512], bf16, tag="wu")
            nc.vector.memset(wu, 0.0)
            wups = psA.tile([P, 512], f32, tag="blk")
            for _ in range(10):
                nc.tensor.matmul(wups, lhsT=wu[:, :P], rhs=wu, start=True, stop=True)

            # ---- load inputs to SBUF (qT deps first) ----
            wq, qx, xT, wv, wk = [], [], [], [], []
            for d in range(4):
                t = cst.tile([P, U], bf16, tag=f"wq{d}")
                nc.sync.dma_start(out=t, in_=wq_d[d * P:(d + 1) * P, :])
                wq.append(t)
                t = cst.tile([P, NSLOT * P], bf16, tag=f"qx{d}")
                nc.sync.dma_start(out=t, in_=qx_d[d * P:(d + 1) * P, :])
                qx.append(t)
            for d in range(4):
                t = cst.tile([P, S], bf16, tag=f"xT{d}")
                xT.append(t)
            for h in range(2):
                for d in range(4):
                    nc.sync.dma_start(
                        out=xT[d][:, h * 1024:(h + 1) * 1024],
                        in_=xT_d[d * P:(d + 1) * P, h * 1024:(h + 1) * 1024])
            for name, dram, lst in (("wv", wv_d, wv), ("wk", wk_d, wk)):
                for d in range(4):
                    t = cst.tile([P, U], bf16, tag=f"{name}{d}")
                    nc.sync.dma_start(out=t, in_=dram[d * P:(d + 1) * P, :])
                    lst.append(t)
            maskmul = cst.tile([P, MTOT], bf16, tag="maskmul")
            nc.sync.dma_start(out=maskmul, in_=mm_d[:, :])
            ident = cst.tile([P, P], bf16, tag="ident")
            nc.sync.dma_start(out=ident, in_=id_d[:, :])
            rsel = cst.tile([1, 2], f32, tag="rsel")
            nc.sync.dma_start(out=rsel, in_=rs_d[:, :])
            sume = cst.tile([P, 1], f32, tag="sume")
            nc.sync.dma_start(out=sume, in_=se_d[:, :])

            # ---- phase 1: qT [u, sq], kT [u, s], v [s, u] ----
            qT = [cst.tile([P, NSLOT * P], bf16, tag=f"qT{u}", name=f"qT{u}")
                  for u in range(4)]
            for u in range(4):
                for h in range(2):
                    ps = psA.tile([P, 512], f32, tag="blk")
                    for d in range(4):
                        nc.tensor.matmul(
                            ps,
                            lhsT=wq[d][:, u * P:(u + 1) * P],
                            rhs=qx[d][:, h * 512:(h + 1) * 512],
                            start=(d == 0), stop=(d == 3),
                        )
                    nc.scalar.copy(qT[u][:, h * 512:(h + 1) * 512], ps)

            kT = [cst.tile([P, S], bf16, tag=f"kT{u}", name=f"kT{u}")
                  for u in range(4)]
            for g in range(4):
                for u in range(4):
                    ps = psA.tile([P, 512], f32, tag="blk")
                    for d in range(4):
                        nc.tensor.matmul(
                            ps,
                            lhsT=wv[d][:, u * P:(u + 1) * P],
                            rhs=xT[d][:, g * 512:(g + 1) * 512],
                            start=(d == 0), stop=(d == 3),
                        )
                    nc.scalar.copy(kT[u][:, g * 512:(g + 1) * 512], ps)

            v_sb = [cst.tile([P, U], bf16, tag=f"v{sc}", name=f"v{sc}")
                    for sc in range(16)]
            for sc in range(16):
                ps = psA.tile([P, 512], f32, tag="blk")
                for d in range(4):
                    nc.tensor.matmul(
                        ps,
                        lhsT=xT[d][:, sc * P:(sc + 1) * P],
                        rhs=wk[d],
                        start=(d == 0), stop=(d == 3),
                    )
                nc.scalar.copy(v_sb[sc], ps)

            # ---- mean-of-v (for the fully-masked global row 0) ----
            xs16 = []
            for d in range(4):
                xs = small.tile([P, 1], f32, tag="xs")
                nc.vector.reduce_sum(xs, xT[d], axis=mybir.AxisListType.X)
                x16 = small.tile([P, 1], bf16, tag="xs16")
                nc.vector.tensor_copy(x16, xs)
                xs16.append(x16)
            vm_ps = psT.tile([1, 512], f32, tag="vm", bufs=1)
            for d in range(4):
                nc.tensor.matmul(vm_ps, lhsT=xs16[d], rhs=wk[d],
                                 start=(d == 0), stop=(d == 3))
            vm_sb = cst.tile([1, 512], f32, tag="vm_sb")
            # vm_sb = sum_s v[s, :] * rscale  (rscale = 1/S for role 0, else 0)
            nc.vector.tensor_scalar_mul(vm_sb, vm_ps, rsel[0:1, 1:2])

            # ---- phase 2: attention per slot ----
            for s in range(NSLOT):
                nb, w = NB[s], W[s]
                ctx_ps = psC.tile([P, 512], f32, tag="ctx")
                bsums = []
                last_c = (nb - 1) * 4 + w // P - 1  # last kk chunk index
                for kb in range(nb):
                    bw = 512 if kb < nb - 1 else w
                    sc_ps = psA.tile([P, 512], f32, tag="blk")
                    for u in range(4):
                        nc.tensor.matmul(
                            sc_ps[:, :bw],
                            lhsT=qT[u][:, s * P:(s + 1) * P],
                            rhs=kT[u][:, kb * 512:kb * 512 + bw],
                            start=(u == 0), stop=(u == 3),
                        )
                    attn = work.tile([P, 512], bf16, tag="attn")
                    bsum = small.tile([P, 1], f32, tag="bsum")
                    if kb == nb - 1:
                        raw = work.tile([P, 512], bf16, tag="raw")
                        nc.scalar.activation(
                            raw[:, :bw], sc_ps[:, :bw],
                            mybir.ActivationFunctionType.Exp, scale=SCALE,
                        )
                        msl = maskmul[:, int(MOFF[s]):int(MOFF[s]) + bw]
                        nc.vector.tensor_mul(attn[:, :bw], raw[:, :bw], msl)
                        nc.vector.reduce_sum(bsum, attn[:, :bw],
                                             axis=mybir.AxisListType.X)
                    else:
                        nc.scalar.activation(
                            attn, sc_ps, mybir.ActivationFunctionType.Exp,
                            scale=SCALE, accum_out=bsum,
                        )
                    bsums.append(bsum)
                    for c in range(bw // P):
                        at_ps = psT.tile([P, P], bf16, tag="at")
                        nc.tensor.transpose(at_ps, attn[:, c * P:(c + 1) * P],
                                            ident)
                        at_sb = work.tile([P, P], bf16, tag="ats")
                        nc.vector.tensor_copy(at_sb, at_ps)
                        cc = kb * 4 + c
                        nc.tensor.matmul(
                            ctx_ps,
                            lhsT=at_sb,
                            rhs=v_sb[cc],
                            start=(cc == 0),
                            stop=(cc == last_c),
                        )
                # combine row sums, reciprocal, normalize, store
                rs = bsums[0]
                for extra in bsums[1:]:
                    nrs = small.tile([P, 1], f32, tag="rs")
                    nc.vector.tensor_add(nrs, rs, extra)
                    rs = nrs
                if s == SPECIAL:
                    nrs = small.tile([P, 1], f32, tag="rs")
                    nc.vector.tensor_add(nrs, rs, sume)
                    rs = nrs
                rcp = small.tile([P, 1], f32, tag="rcp")
                nc.vector.reciprocal(rcp, rs)
                ctx_sb = work.tile([P, 512], f32, tag="ctxs")
                nc.scalar.activation(
                    ctx_sb, ctx_ps, mybir.ActivationFunctionType.Copy,
                    scale=rcp,
                )
                if s == SPECIAL:
                    # row 0 of role 0 = mean(v): ctx*rsel + sum(v)*rscale
                    nc.vector.tensor_scalar_mul(ctx_sb[0:1, :], ctx_sb[0:1, :],
                                                rsel[0:1, 0:1])
                    nc.vector.tensor_add(ctx_sb[0:1, :], ctx_sb[0:1, :], vm_sb)
                nc.sync.dma_start(out=out_d[s * P:(s + 1) * P, :], in_=ctx_sb)

    nc.compile()
    _nc_cache = nc
    return nc


def host_inputs(query, Wq, Wv, Wk):
    """Build per-core input maps. query [B,S,D] f32; W* [D,U] f32."""
    wq16 = Wq.astype(BF16)
    wv16 = Wv.astype(BF16)
    wk16 = Wk.astype(BF16)
    ident = np.eye(P, dtype=BF16)

    i = np.arange(P)[:, None]
    masks = {}
    for r in range(2):
        mm = np.zeros((P, MTOT), np.float32)
        for s in range(NSLOT):
            t = SLOT_TILES[s] + r
            kb = NB[s] - 1
            j = np.arange(W[s])[None, :]
            mm[:, MOFF[s]:MOFF[s] + W[s]] = (512 * kb + j < 128 * t + i)
        masks[r] = mm.astype(BF16)

    in_maps = []
    for core in range(8):
        b, r = core // 2, core % 2
        xTb = np.ascontiguousarray(query[b].T).astype(BF16)       # [D, S]
        cols = np.concatenate(
            [np.arange(128 * (SLOT_TILES[s] + r), 128 * (SLOT_TILES[s] + r) + P)
             for s in range(NSLOT)]
        )
        qx = np.ascontiguousarray(xTb[:, cols])                    # [D, 1024]
        rsel = np.array([[0.0, 1.0 / S]] if r == 0 else [[1.0, 0.0]], np.float32)
        sume = np.zeros((P, 1), np.float32)
        if r == 0:
            sume[0, 0] = 1.0  # avoid 1/0 on the fully-masked row
        in_maps.append({
            "xT": xTb, "qx": qx,
            "wq": wq16, "wv": wv16, "wk": wk16,
            "maskmul": masks[r], "ident": ident,
            "rsel": rsel, "sume": sume,
        })
    return in_maps


def assemble_output(results):
    """results: list of 8 dicts with 'out' [1024, 512] f32."""
    out = np.zeros((B, S, U), np.float32)
    for core in range(8):
        b, r = core // 2, core % 2
        o = np.asarray(results[core]["out"], dtype=np.float32)
        for s in range(NSLOT):
            t = SLOT_TILES[s] + r
            out[b, 128 * t:128 * (t + 1), :] = o[128 * s:128 * (s + 1), :]
    return out


def run(query, Wq, Wv, Wk, **kwargs):
    """Build, compile, and execute on all 8 cores. Returns (output, results)."""
    nc = build_nc()
    in_maps = host_inputs(
        np.asarray(query, np.float32), np.asarray(Wq, np.float32),
        np.asarray(Wv, np.float32), np.asarray(Wk, np.float32),
    )
    res = bass_utils.run_bass_kernel_spmd(nc, in_maps, list(range(8)), **kwargs)
    return assemble_output(res.results), res


def kernel(query, Wq, Wv, Wk):
    out, _ = run(query, Wq, Wv, Wk)
    return out


if __name__ == "__main__":
    rng = np.random.default_rng(0)
    q = rng.standard_normal((B, S, D), dtype=np.float32)
    scale = np.sqrt(2.0 / (D + U)).astype(np.float32)
    Wq = rng.standard_normal((D, U), dtype=np.float32) * scale
    Wv = rng.standard_normal((D, U), dtype=np.float32) * scale
    Wk = rng.standard_normal((D, U), dtype=np.float32) * scale
    out = kernel(q, Wq, Wv, Wk)
    print(out.shape, out.dtype, np.abs(out).mean())
